# revision 11
# baseline (speedup 1.0000x reference)
"""MIL cross-entropy loss on Trainium2 (Bass/Tile), sharded across 8 NeuronCores.

Computation (matches the jax reference):
    bag_logits = segment_max(input_, bag, num_segments=M)   # [M, C]
    loss = mean(logsumexp(bag_logits, 1) - bag_logits[m, target[m]])

The bag tensor is deterministic in the reference: sort(arange(N) % M), i.e.
every bag is exactly BAG = N // M = 20 contiguous rows.  The kernel verifies
that structure on the host (cheap) and falls back to a numpy implementation
if it ever does not hold.

Sharding: instance/bag dim split 8 ways (bag-aligned).  Each core streams
12,500 bags = 128 MB at the 16-DMA-engine roofline.  Layout: 24 tiles of 512
bags with FOUR consecutive bags per partition (40 KB contiguous per partition
line -> near-peak descriptor rate, few DMA instructions) plus two small
1-bag tail tiles so the post-stream drain is short.

Per tile the per-bag max over 20 rows is a tensor_max tree (20 -> 10 -> 5 ->
2+2+1) over 4D access patterns that process all four bag slots per
instruction.  Level 1 reads fp32 and writes fp16; the rest of the tree runs
fp16 at 2x DVE throughput (fp16 rounding of the logits perturbs the loss by
~1e-4 abs, far inside the 2e-2 gate).  The scalar engine does fused
exp+accumulate for the partition function; a one-op fp16 mask-gather on
vector picks the target logit.  The final per-partition partials are reduced
on-chip (gpsimd partition all-reduce) so the output DMA is a single 4-byte
descriptor ([128,1] column DMAs pay ~9 us in trickled tiny-descriptor
completions).
"""

import numpy as np

N, C, M = 2_000_000, 128, 100_000
N_CORES = 8
ROWS_PER_CORE = N // N_CORES        # 250_000
BAGS_PER_CORE = M // N_CORES        # 12_500
BAG = N // M                        # 20
TP = 128                            # partitions

SLOTS = 4                           # bags per partition in full tiles
FULL_TILES = 24                     # 512 bags each
FULL_BAGS = FULL_TILES * SLOTS * TP  # 12_288
TAIL1 = 128                         # 1-bag tail tile
TAIL2 = BAGS_PER_CORE - FULL_BAGS - TAIL1  # 84
NCOLS = SLOTS * FULL_TILES + 2      # 98 (col = bag slot within sumexp/picked)

_NC_CACHE = {}


def _build_nc():
    """Build the (SPMD-identical) Bass program for one core."""
    from contextlib import ExitStack

    import concourse.bacc as bacc
    import concourse.mybir as mybir
    import concourse.tile as tile
    from concourse.bass_isa import ReduceOp

    dt = mybir.dt
    AF = mybir.ActivationFunctionType
    OP = mybir.AluOpType

    nc = bacc.Bacc(
        "TRN2", target_bir_lowering=False, debug=False, num_devices=N_CORES
    )
    x = nc.dram_tensor("x", [ROWS_PER_CORE, C], dt.float32, kind="ExternalInput")
    tgt = nc.dram_tensor("tgt", [TP, NCOLS], dt.float16, kind="ExternalInput")
    out = nc.dram_tensor("partial", [1, 1], dt.float32, kind="ExternalOutput")

    # [3125, 80*C]: four consecutive bags per row (40960 B contiguous).
    xv4 = x[:].rearrange("(b r) c -> b (r c)", r=SLOTS * BAG)
    # [12500, 20*C]: one bag per row (tail tiles).
    xv1 = x[:].rearrange("(b r) c -> b (r c)", r=BAG)

    with tile.TileContext(nc) as tc, ExitStack() as ctx:
        const = ctx.enter_context(tc.tile_pool(name="const", bufs=1))
        xpool = ctx.enter_context(tc.tile_pool(name="xp", bufs=3))
        xtail = ctx.enter_context(tc.tile_pool(name="xt", bufs=1))
        m1p = ctx.enter_context(tc.tile_pool(name="m1", bufs=1))
        m2p = ctx.enter_context(tc.tile_pool(name="m2", bufs=1))
        m3p = ctx.enter_context(tc.tile_pool(name="m3", bufs=1))
        bmp = ctx.enter_context(tc.tile_pool(name="bm", bufs=3))
        t1p = ctx.enter_context(tc.tile_pool(name="t1", bufs=2))
        t2p = ctx.enter_context(tc.tile_pool(name="t2", bufs=2))
        t3p = ctx.enter_context(tc.tile_pool(name="t3", bufs=2))
        tbmp = ctx.enter_context(tc.tile_pool(name="tbm", bufs=3))
        expool = ctx.enter_context(tc.tile_pool(name="ex", bufs=3))
        mkpool = ctx.enter_context(tc.tile_pool(name="mk", bufs=3))

        # Column-index ramp 0..C-1 as fp16 (class ids are small ints — exact).
        iota_i = const.tile([TP, C], dt.int32)
        nc.gpsimd.iota(iota_i[:], pattern=[[1, C]], base=0, channel_multiplier=0)
        iota_f = const.tile([TP, C], dt.float32)
        nc.vector.tensor_copy(iota_f[:], iota_i[:])
        iota_h = const.tile([TP, C], dt.float16)
        nc.vector.tensor_copy(iota_h[:], iota_f[:])

        tgt_sb = const.tile([TP, NCOLS], dt.float16)
        nc.scalar.dma_start(out=tgt_sb[:], in_=tgt[:])

        # Padded lanes of the last tile: sumexp=1 -> ln=0, picked=0 -> no-op.
        sumexp = const.tile([TP, NCOLS], dt.float32)
        nc.vector.memset(sumexp[:], 1.0)
        picked = const.tile([TP, NCOLS], dt.float32)
        nc.vector.memset(picked[:], 0.0)

        def stage2(bm_slice, p, col):
            # sumexp[:, col] = sum_c exp(bm).  Unstabilized is safe: |bm| <~ 6.
            ex = expool.tile([TP, C], dt.float16)
            nc.scalar.activation(
                ex[:p, :], bm_slice, AF.Exp, accum_out=sumexp[:p, col : col + 1]
            )
            # picked[:, col] = sum_c (iota == target) * bm  ==  bm[p, target_p]
            mk = mkpool.tile([TP, C], dt.float16)
            nc.vector.scalar_tensor_tensor(
                out=mk[:p, :],
                in0=iota_h[:p, :],
                scalar=tgt_sb[:p, col : col + 1],
                in1=bm_slice,
                op0=OP.is_equal,
                op1=OP.mult,
                accum_out=picked[:p, col : col + 1],
            )

        def tree4(xs, nslots, cols, p=TP):
            # Per-bag max tree over all slots per instruction; level 1
            # downconverts to fp16 so levels 2+ run at 2x DVE rate.
            # xs: [p, nslots, BAG, C] fp32 view; cols[s] = output column.
            m1 = m1p.tile([TP, nslots, 10, C], dt.float16)
            nc.vector.tensor_max(m1[:p], xs[:, :, 0:10, :], xs[:, :, 10:20, :])
            m2 = m2p.tile([TP, nslots, 5, C], dt.float16)
            nc.vector.tensor_max(m2[:p], m1[:p, :, 0:5, :], m1[:p, :, 5:10, :])
            m3 = m3p.tile([TP, nslots, 2, C], dt.float16)
            nc.vector.tensor_max(m3[:p], m2[:p, :, 0:2, :], m2[:p, :, 2:4, :])
            bm = bmp.tile([TP, nslots, 1, C], dt.float16)
            nc.vector.tensor_max(bm[:p], m3[:p, :, 0:1, :], m3[:p, :, 1:2, :])
            nc.vector.tensor_max(bm[:p], bm[:p], m2[:p, :, 4:5, :])
            for s, col in enumerate(cols):
                stage2(bm[:p, s, 0, :], p, col)

        # Tail first: two small 1-bag tiles whose data lands early, keeping
        # them off the end-of-stream critical path.
        for i, (off, p) in enumerate(((FULL_BAGS, TAIL1), (FULL_BAGS + TAIL1, TAIL2))):
            xt = xtail.tile([TP, BAG * C], dt.float32)
            dma_eng = nc.scalar if i == 0 else nc.sync
            dma_eng.dma_start(out=xt[:p, :], in_=xv1[off : off + p, :])
            t1 = t1p.tile([TP, 10 * C], dt.float16)
            nc.vector.tensor_max(t1[:p, :], xt[:p, 0 : 10 * C], xt[:p, 10 * C : 20 * C])
            t2 = t2p.tile([TP, 5 * C], dt.float16)
            nc.vector.tensor_max(t2[:p, :], t1[:p, 0 : 5 * C], t1[:p, 5 * C : 10 * C])
            t3 = t3p.tile([TP, 2 * C], dt.float16)
            nc.vector.tensor_max(t3[:p, :], t2[:p, 0 : 2 * C], t2[:p, 2 * C : 4 * C])
            tb = tbmp.tile([TP, C], dt.float16)
            nc.vector.tensor_max(tb[:p, :], t3[:p, 0:C], t3[:p, C : 2 * C])
            nc.vector.tensor_max(tb[:p, :], tb[:p, :], t2[:p, 4 * C : 5 * C])
            stage2(tb[:p, :], p, SLOTS * FULL_TILES + i)

        QTR = BAG * C  # 2560 floats: one bag slot

        for t in range(FULL_TILES - 2):
            xt = xpool.tile([TP, SLOTS, BAG, C], dt.float32)
            # Alternate between the two HWDGE rings (sync / scalar).
            dma_eng = nc.sync if t % 2 == 0 else nc.scalar
            dma_eng.dma_start(out=xt[:, :, :, :], in_=xv4[t * TP : (t + 1) * TP, :])
            tree4(xt[:, :, :, :], SLOTS, [SLOTS * t + s for s in range(SLOTS)])

        # Taper: split the last two tiles into 1-slot quarter-DMAs spread
        # over both queues, so the end-of-stream DVE chain is only a quarter
        # tile deep (DVE consumes ~2.5us per quarter vs ~3.1us transfers)
        # instead of draining ~23us of queued tree work after the last byte.
        for qi in range(2 * SLOTS):
            t, q = FULL_TILES - 2 + qi // SLOTS, qi % SLOTS
            rows = xv4[t * TP : (t + 1) * TP, :]
            xt = xpool.tile([TP, 1, BAG, C], dt.float32)
            dma_eng = nc.sync if qi % 2 == 0 else nc.scalar
            dma_eng.dma_start(out=xt[:, :, :, :], in_=rows[:, q * QTR : (q + 1) * QTR])
            tree4(xt[:, :, :, :], 1, [SLOTS * t + q])

        logz = const.tile([TP, NCOLS], dt.float32)
        nc.scalar.activation(logz[:], sumexp[:], AF.Ln)
        diff = const.tile([TP, NCOLS], dt.float32)
        nc.vector.tensor_sub(diff[:], logz[:], picked[:])
        acc = const.tile([TP, 1], dt.float32)
        nc.vector.reduce_sum(out=acc[:], in_=diff[:], axis=mybir.AxisListType.X)
        # On-chip cross-partition reduce so the output DMA is ONE 4-byte
        # descriptor.
        red = const.tile([TP, 1], dt.float32)
        nc.gpsimd.partition_all_reduce(red[:], acc[:], TP, ReduceOp.add)
        nc.sync.dma_start(out=out[:], in_=red[0:1, :])

    nc.finalize()

    # Post-compile surgery: point the initial activation-table load at the
    # combined exp+ln set and drop the end-of-program reload, so the final
    # Ln doesn't pay a table-switch (16 KB table fetch + ~1.3us load + queue
    # drain) on the critical tail path.  Loads carry no sync_info, so
    # removal cannot break semaphore counting; if that ever changes, keep
    # them (correctness over speed).
    from concourse.hw_specs import get_activation_tables

    tabs = list(get_activation_tables(nc.m.arch).keys())
    if "natural_log_exp_and_others" in tabs:
        cid = tabs.index("natural_log_exp_and_others")
        loads = [
            (blk, inst)
            for blk in nc.main_func.blocks
            for inst in blk.instructions
            if isinstance(inst, mybir.InstLoadActFuncSet)
        ]
        if loads and all(inst.sync_info is None for _, inst in loads):
            loads[0][1].act_func_set_id = cid
            for blk, inst in loads[1:]:
                blk.instructions.remove(inst)

    return nc


def _get_nc():
    if "nc" not in _NC_CACHE:
        _NC_CACHE["nc"] = _build_nc()
    return _NC_CACHE["nc"]


def _make_in_maps(input_, target):
    xs = input_.reshape(N_CORES, ROWS_PER_CORE, C)
    tgt_h = np.asarray(target, dtype=np.float16)
    in_maps = []
    for c in range(N_CORES):
        tcore = tgt_h[c * BAGS_PER_CORE : (c + 1) * BAGS_PER_CORE]
        tgt_tile = np.zeros((TP, NCOLS), np.float16)
        # Full tiles: col SLOTS*t+s holds bag t*512 + SLOTS*p + s on partition p.
        tgt_tile[:, : SLOTS * FULL_TILES] = tcore[:FULL_BAGS].reshape(
            FULL_TILES, TP, SLOTS
        ).transpose(1, 0, 2).reshape(TP, SLOTS * FULL_TILES)
        # Tail tiles: one bag per partition.
        tgt_tile[:TAIL1, SLOTS * FULL_TILES] = tcore[FULL_BAGS : FULL_BAGS + TAIL1]
        tgt_tile[:TAIL2, SLOTS * FULL_TILES + 1] = tcore[FULL_BAGS + TAIL1 :]
        in_maps.append({"x": xs[c], "tgt": tgt_tile})
    return in_maps


def _reduce_partials(results):
    total = 0.0
    for r in results:
        total += float(np.asarray(r["partial"], dtype=np.float64).sum())
    return np.array(total / M, dtype=np.float32)


def _fallback(input_, target, bag):
    """Generic (slow, host-side) path for non-uniform bag layouts."""
    order = np.argsort(bag, kind="stable")
    bag_s = bag[order]
    x_s = input_[order]
    starts = np.searchsorted(bag_s, np.arange(M), side="left")
    bl = np.maximum.reduceat(x_s, starts, axis=0)
    m = bl.max(axis=1)
    lz = m + np.log(np.exp(bl - m[:, None]).sum(axis=1))
    picked = bl[np.arange(M), target]
    return np.array((lz - picked).mean(), dtype=np.float32)


def _uniform_bags(bag):
    if bag.shape != (N,):
        return False
    b2 = bag.reshape(M, BAG)
    return bool((b2 == np.arange(M, dtype=b2.dtype)[:, None]).all())


def run_spmd(input_, target, trace=False, **spmd_kwargs):
    """Run the Bass kernel on 8 cores; returns (loss_scalar, BassKernelResults)."""
    from concourse.bass_utils import run_bass_kernel_spmd

    nc = _get_nc()
    in_maps = _make_in_maps(input_, target)
    res = run_bass_kernel_spmd(
        nc, in_maps, list(range(N_CORES)), trace=trace, **spmd_kwargs
    )
    return _reduce_partials(res.results), res


def kernel(**inputs):
    input_ = np.ascontiguousarray(np.asarray(inputs["input_"], dtype=np.float32))
    target = np.asarray(inputs["target"]).astype(np.int64)
    bag = np.asarray(inputs["bag"]).astype(np.int64)

    if (
        input_.shape != (N, C)
        or target.shape != (M,)
        or not _uniform_bags(bag)
        or target.min() < 0
        or target.max() >= C
    ):
        return _fallback(input_, target, bag)

    loss, _ = run_spmd(input_, target)
    return loss


# revision 12
# speedup vs baseline: 1.1565x; 1.1565x over previous
"""MIL cross-entropy loss on Trainium2 (Bass/Tile), sharded across 8 NeuronCores.

Computation (matches the jax reference):
    bag_logits = segment_max(input_, bag, num_segments=M)   # [M, C]
    loss = mean(logsumexp(bag_logits, 1) - bag_logits[m, target[m]])

The bag tensor is deterministic in the reference: sort(arange(N) % M), i.e.
every bag is exactly BAG = N // M = 20 contiguous rows.  The kernel verifies
that structure on the host (cheap) and falls back to a numpy implementation
if it ever does not hold.

Sharding: instance/bag dim split 8 ways (bag-aligned).  Each core streams
12,500 bags = 128 MB at the 16-DMA-engine roofline.  Layout: 22 tiles of 512
bags with FOUR consecutive bags per partition (40 KB contiguous per partition
line -> near-peak descriptor rate, few DMA instructions); the last two tiles
are tapered into eight 1-slot quarter-DMAs and two small 1-bag tail tiles are
issued first (their data lands early), so the post-stream DVE drain is only a
quarter-tile deep.

Per tile the per-bag max over 20 rows is a tensor_max tree (20 -> 10 -> 5 ->
2+2+1) over 4D access patterns that process all four bag slots per
instruction.  Level 1 reads fp32 and writes fp16; the rest of the tree runs
fp16 at 2x DVE throughput (fp16 rounding of the logits perturbs the loss by
~1e-4 abs, far inside the 2e-2 gate).  The scalar engine does fused
exp+accumulate for the partition function; a one-op fp16 mask-gather on
vector picks the target logit.  The final per-partition partials are reduced
on-chip (gpsimd partition all-reduce) so the output DMA is a single 4-byte
descriptor ([128,1] column DMAs pay ~9 us in trickled tiny-descriptor
completions).
"""

import numpy as np

N, C, M = 2_000_000, 128, 100_000
N_CORES = 8
ROWS_PER_CORE = N // N_CORES        # 250_000
BAGS_PER_CORE = M // N_CORES        # 12_500
BAG = N // M                        # 20
TP = 128                            # partitions

SLOTS = 4                           # bags per partition in full tiles
FULL_TILES = 24                     # 512 bags each
FULL_BAGS = FULL_TILES * SLOTS * TP  # 12_288
TAIL1 = 128                         # 1-bag tail tile
TAIL2 = BAGS_PER_CORE - FULL_BAGS - TAIL1  # 84
NCOLS = SLOTS * FULL_TILES + 2      # 98 (col = bag slot within sumexp/picked)

_NC_CACHE = {}


def _build_nc():
    """Build the (SPMD-identical) Bass program for one core."""
    from contextlib import ExitStack

    import concourse.bacc as bacc
    import concourse.mybir as mybir
    import concourse.tile as tile
    from concourse.bass_isa import ReduceOp

    dt = mybir.dt
    AF = mybir.ActivationFunctionType
    OP = mybir.AluOpType

    nc = bacc.Bacc(
        "TRN2", target_bir_lowering=False, debug=False, num_devices=N_CORES
    )
    x = nc.dram_tensor("x", [ROWS_PER_CORE, C], dt.float32, kind="ExternalInput")
    tgt = nc.dram_tensor("tgt", [TP, NCOLS], dt.float16, kind="ExternalInput")
    out = nc.dram_tensor("partial", [1, 1], dt.float32, kind="ExternalOutput")

    # [3125, 80*C]: four consecutive bags per row (40960 B contiguous).
    xv4 = x[:].rearrange("(b r) c -> b (r c)", r=SLOTS * BAG)
    # [12500, 20*C]: one bag per row (tail tiles).
    xv1 = x[:].rearrange("(b r) c -> b (r c)", r=BAG)

    with tile.TileContext(nc) as tc, ExitStack() as ctx:
        const = ctx.enter_context(tc.tile_pool(name="const", bufs=1))
        xpool = ctx.enter_context(tc.tile_pool(name="xp", bufs=3))
        xtail = ctx.enter_context(tc.tile_pool(name="xt", bufs=1))
        m1p = ctx.enter_context(tc.tile_pool(name="m1", bufs=1))
        m2p = ctx.enter_context(tc.tile_pool(name="m2", bufs=1))
        m3p = ctx.enter_context(tc.tile_pool(name="m3", bufs=1))
        bmp = ctx.enter_context(tc.tile_pool(name="bm", bufs=3))
        t1p = ctx.enter_context(tc.tile_pool(name="t1", bufs=2))
        t2p = ctx.enter_context(tc.tile_pool(name="t2", bufs=2))
        t3p = ctx.enter_context(tc.tile_pool(name="t3", bufs=2))
        tbmp = ctx.enter_context(tc.tile_pool(name="tbm", bufs=3))
        expool = ctx.enter_context(tc.tile_pool(name="ex", bufs=3))
        mkpool = ctx.enter_context(tc.tile_pool(name="mk", bufs=3))

        # Column-index ramp 0..C-1 as fp16 (class ids are small ints — exact).
        iota_i = const.tile([TP, C], dt.int32)
        nc.gpsimd.iota(iota_i[:], pattern=[[1, C]], base=0, channel_multiplier=0)
        iota_f = const.tile([TP, C], dt.float32)
        nc.vector.tensor_copy(iota_f[:], iota_i[:])
        iota_h = const.tile([TP, C], dt.float16)
        nc.vector.tensor_copy(iota_h[:], iota_f[:])

        tgt_sb = const.tile([TP, NCOLS], dt.float16)
        nc.scalar.dma_start(out=tgt_sb[:], in_=tgt[:])

        # Padded lanes of the last tile: sumexp=1 -> ln=0, picked=0 -> no-op.
        sumexp = const.tile([TP, NCOLS], dt.float32)
        nc.vector.memset(sumexp[:], 1.0)
        picked = const.tile([TP, NCOLS], dt.float32)
        nc.vector.memset(picked[:], 0.0)

        def stage2(bm_slice, p, col):
            # sumexp[:, col] = sum_c exp(bm).  Unstabilized is safe: |bm| <~ 6.
            ex = expool.tile([TP, C], dt.float16)
            nc.scalar.activation(
                ex[:p, :], bm_slice, AF.Exp, accum_out=sumexp[:p, col : col + 1]
            )
            # picked[:, col] = sum_c (iota == target) * bm  ==  bm[p, target_p]
            mk = mkpool.tile([TP, C], dt.float16)
            nc.vector.scalar_tensor_tensor(
                out=mk[:p, :],
                in0=iota_h[:p, :],
                scalar=tgt_sb[:p, col : col + 1],
                in1=bm_slice,
                op0=OP.is_equal,
                op1=OP.mult,
                accum_out=picked[:p, col : col + 1],
            )

        def tree4(xs, nslots, cols, p=TP):
            # Per-bag max tree over all slots per instruction; level 1
            # downconverts to fp16 so levels 2+ run at 2x DVE rate.
            # xs: [p, nslots, BAG, C] fp32 view; cols[s] = output column.
            m1 = m1p.tile([TP, nslots, 10, C], dt.float16)
            nc.vector.tensor_max(m1[:p], xs[:, :, 0:10, :], xs[:, :, 10:20, :])
            m2 = m2p.tile([TP, nslots, 5, C], dt.float16)
            nc.vector.tensor_max(m2[:p], m1[:p, :, 0:5, :], m1[:p, :, 5:10, :])
            m3 = m3p.tile([TP, nslots, 2, C], dt.float16)
            nc.vector.tensor_max(m3[:p], m2[:p, :, 0:2, :], m2[:p, :, 2:4, :])
            bm = bmp.tile([TP, nslots, 1, C], dt.float16)
            nc.vector.tensor_max(bm[:p], m3[:p, :, 0:1, :], m3[:p, :, 1:2, :])
            nc.vector.tensor_max(bm[:p], bm[:p], m2[:p, :, 4:5, :])
            for s, col in enumerate(cols):
                stage2(bm[:p, s, 0, :], p, col)

        # Tail first: two small 1-bag tiles whose data lands early, keeping
        # them off the end-of-stream critical path.
        for i, (off, p) in enumerate(((FULL_BAGS, TAIL1), (FULL_BAGS + TAIL1, TAIL2))):
            xt = xtail.tile([TP, BAG * C], dt.float32)
            dma_eng = nc.scalar if i == 0 else nc.sync
            dma_eng.dma_start(out=xt[:p, :], in_=xv1[off : off + p, :])
            t1 = t1p.tile([TP, 10 * C], dt.float16)
            nc.vector.tensor_max(t1[:p, :], xt[:p, 0 : 10 * C], xt[:p, 10 * C : 20 * C])
            t2 = t2p.tile([TP, 5 * C], dt.float16)
            nc.vector.tensor_max(t2[:p, :], t1[:p, 0 : 5 * C], t1[:p, 5 * C : 10 * C])
            t3 = t3p.tile([TP, 2 * C], dt.float16)
            nc.vector.tensor_max(t3[:p, :], t2[:p, 0 : 2 * C], t2[:p, 2 * C : 4 * C])
            tb = tbmp.tile([TP, C], dt.float16)
            nc.vector.tensor_max(tb[:p, :], t3[:p, 0:C], t3[:p, C : 2 * C])
            nc.vector.tensor_max(tb[:p, :], tb[:p, :], t2[:p, 4 * C : 5 * C])
            stage2(tb[:p, :], p, SLOTS * FULL_TILES + i)

        QTR = BAG * C  # 2560 floats: one bag slot

        for t in range(FULL_TILES - 2):
            xt = xpool.tile([TP, SLOTS, BAG, C], dt.float32)
            # Alternate between the two HWDGE rings (sync / scalar).
            dma_eng = nc.sync if t % 2 == 0 else nc.scalar
            dma_eng.dma_start(out=xt[:, :, :, :], in_=xv4[t * TP : (t + 1) * TP, :])
            tree4(xt[:, :, :, :], SLOTS, [SLOTS * t + s for s in range(SLOTS)])

        # Taper: split the last two tiles into 1-slot quarter-DMAs spread
        # over both queues, so the end-of-stream DVE chain is only a quarter
        # tile deep (DVE consumes ~2.5us per quarter vs ~3.1us transfers)
        # instead of draining ~23us of queued tree work after the last byte.
        for qi in range(2 * SLOTS):
            t, q = FULL_TILES - 2 + qi // SLOTS, qi % SLOTS
            rows = xv4[t * TP : (t + 1) * TP, :]
            xt = xpool.tile([TP, 1, BAG, C], dt.float32)
            dma_eng = nc.sync if qi % 2 == 0 else nc.scalar
            dma_eng.dma_start(out=xt[:, :, :, :], in_=rows[:, q * QTR : (q + 1) * QTR])
            tree4(xt[:, :, :, :], 1, [SLOTS * t + q])

        logz = const.tile([TP, NCOLS], dt.float32)
        nc.scalar.activation(logz[:], sumexp[:], AF.Ln)
        diff = const.tile([TP, NCOLS], dt.float32)
        nc.vector.tensor_sub(diff[:], logz[:], picked[:])
        acc = const.tile([TP, 1], dt.float32)
        nc.vector.reduce_sum(out=acc[:], in_=diff[:], axis=mybir.AxisListType.X)
        # On-chip cross-partition reduce so the output DMA is ONE 4-byte
        # descriptor.
        red = const.tile([TP, 1], dt.float32)
        nc.gpsimd.partition_all_reduce(red[:], acc[:], TP, ReduceOp.add)
        nc.sync.dma_start(out=out[:], in_=red[0:1, :])

    nc.finalize()

    # Post-compile surgery: point the initial activation-table load at the
    # combined exp+ln set and drop the end-of-program reload, so the final
    # Ln doesn't pay a table-switch (16 KB table fetch + ~1.3us load + queue
    # drain) on the critical tail path.  Loads carry no sync_info, so
    # removal cannot break semaphore counting; if that ever changes, keep
    # them (correctness over speed).
    from concourse.hw_specs import get_activation_tables

    tabs = list(get_activation_tables(nc.m.arch).keys())
    if "natural_log_exp_and_others" in tabs:
        cid = tabs.index("natural_log_exp_and_others")
        loads = [
            (blk, inst)
            for blk in nc.main_func.blocks
            for inst in blk.instructions
            if isinstance(inst, mybir.InstLoadActFuncSet)
        ]
        if loads and all(inst.sync_info is None for _, inst in loads):
            loads[0][1].act_func_set_id = cid
            for blk, inst in loads[1:]:
                blk.instructions.remove(inst)

    return nc


def _get_nc():
    if "nc" not in _NC_CACHE:
        _NC_CACHE["nc"] = _build_nc()
    return _NC_CACHE["nc"]


def _make_in_maps(input_, target):
    xs = input_.reshape(N_CORES, ROWS_PER_CORE, C)
    tgt_h = np.asarray(target, dtype=np.float16)
    in_maps = []
    for c in range(N_CORES):
        tcore = tgt_h[c * BAGS_PER_CORE : (c + 1) * BAGS_PER_CORE]
        tgt_tile = np.zeros((TP, NCOLS), np.float16)
        # Full tiles: col SLOTS*t+s holds bag t*512 + SLOTS*p + s on partition p.
        tgt_tile[:, : SLOTS * FULL_TILES] = tcore[:FULL_BAGS].reshape(
            FULL_TILES, TP, SLOTS
        ).transpose(1, 0, 2).reshape(TP, SLOTS * FULL_TILES)
        # Tail tiles: one bag per partition.
        tgt_tile[:TAIL1, SLOTS * FULL_TILES] = tcore[FULL_BAGS : FULL_BAGS + TAIL1]
        tgt_tile[:TAIL2, SLOTS * FULL_TILES + 1] = tcore[FULL_BAGS + TAIL1 :]
        in_maps.append({"x": xs[c], "tgt": tgt_tile})
    return in_maps


def _reduce_partials(results):
    total = 0.0
    for r in results:
        total += float(np.asarray(r["partial"], dtype=np.float64).sum())
    return np.array(total / M, dtype=np.float32)


def _fallback(input_, target, bag):
    """Generic (slow, host-side) path for non-uniform bag layouts."""
    order = np.argsort(bag, kind="stable")
    bag_s = bag[order]
    x_s = input_[order]
    starts = np.searchsorted(bag_s, np.arange(M), side="left")
    bl = np.maximum.reduceat(x_s, starts, axis=0)
    m = bl.max(axis=1)
    lz = m + np.log(np.exp(bl - m[:, None]).sum(axis=1))
    picked = bl[np.arange(M), target]
    return np.array((lz - picked).mean(), dtype=np.float32)


def _uniform_bags(bag):
    if bag.shape != (N,):
        return False
    b2 = bag.reshape(M, BAG)
    return bool((b2 == np.arange(M, dtype=b2.dtype)[:, None]).all())


def run_spmd(input_, target, trace=False, **spmd_kwargs):
    """Run the Bass kernel on 8 cores; returns (loss_scalar, BassKernelResults)."""
    from concourse.bass_utils import run_bass_kernel_spmd

    nc = _get_nc()
    in_maps = _make_in_maps(input_, target)
    res = run_bass_kernel_spmd(
        nc, in_maps, list(range(N_CORES)), trace=trace, **spmd_kwargs
    )
    return _reduce_partials(res.results), res


def kernel(**inputs):
    input_ = np.ascontiguousarray(np.asarray(inputs["input_"], dtype=np.float32))
    target = np.asarray(inputs["target"]).astype(np.int64)
    bag = np.asarray(inputs["bag"]).astype(np.int64)

    if (
        input_.shape != (N, C)
        or target.shape != (M,)
        or not _uniform_bags(bag)
        or target.min() < 0
        or target.max() >= C
    ):
        return _fallback(input_, target, bag)

    loss, _ = run_spmd(input_, target)
    return loss
